# revision 13
# baseline (speedup 1.0000x reference)
"""Trainium2 Bass kernel for BlockSoftmaxLinearHybrid.

The warm-call wall time is dominated by the axon tunnel, whose
bandwidth is limited PER CLIENT CONNECTION (~35MB/s each way today,
but N independent client processes scale to ~N x that).  So the host
side runs 8 persistent worker processes, each with its own axon client
session and one NeuronCore, each handling 4 of the 32 (b,h) pairs:

  * the parent quantizes q/k/v rows to int8 (+ per-row bf16 scale in
    the trailing 2 bytes) directly into shared memory, round-robin one
    pair at a time so all 8 connections go busy almost immediately;
  * each worker uploads its pairs as they arrive, runs the Bass kernel
    (one exec per pair), downloads the int8 output and dequantizes it
    into a shared f32 output buffer;
  * emulated end-to-end rel-err with q/k/v and the output all at int8
    is 0.0152 vs the 0.02 gate.

If worker spawn/init fails, falls back to a single-process 8-core
shard_map path, then to a host numpy reference.

Device kernel per (b,h) pair:
  phase A: dequantize q/k, transpose both on the PE array to D-major;
           u_q^T = W^T Q^T (f-major), EXPQ=[exp(u);exp(-u)] unnormalized
           (normalization recovered via ones-column in the state matmul);
           u_k in natural layout, exp'd and row-normalized -> phi_k.
  phase B: per 64-row block scan: block-local softmax attention
           (scores^T -> exp -> @[v|1]) + linear attention vs the running
           [S|Z] state accumulated in PSUM, blended with w=sigmoid(alpha).
"""

import os
import sys
import threading
import time

import numpy as np

if "/opt/trn_rl_repo" not in sys.path:
    sys.path.insert(0, "/opt/trn_rl_repo")
_KDIR = os.path.dirname(os.path.abspath(__file__))
if _KDIR not in sys.path:
    sys.path.insert(0, _KDIR)

import ml_dtypes

import concourse.bass as bass
import concourse.bacc as bacc
import concourse.mybir as mybir
from concourse.tile import TileContext

B, H, L, D = 2, 16, 4096, 128
F = 64          # feature dim; phi dim is 2F = 128
SBLK = 64       # block size
NBLK = L // SBLK            # 64 blocks
NCH = L // 128              # 32 chunks (2 blocks each)
EPS = 1e-6
SCALING = D ** -0.5
NCORES = 8
PAIRS = B * H               # 32 (b,h) pairs
NCALL = 4                   # fallback path: pipeline chunks per call
RPC = PAIRS // NCALL        # fallback: pairs per chunk
PPC = RPC // NCORES         # pairs per core per exec (= 1)
NGRP = NCH                  # phase-B group count
NW = 8                      # worker processes (one per core)

BF16 = mybir.dt.bfloat16
F16 = mybir.dt.float16
F32 = mybir.dt.float32
INT8 = mybir.dt.int8
AX = mybir.AxisListType
ALU = mybir.AluOpType
ACTF = mybir.ActivationFunctionType
BFDT = ml_dtypes.bfloat16


def _bcast_last(ap, n):
    """Append a stride-0 dim of size n to an AP (free-dim broadcast)."""
    return bass.AP(tensor=ap.tensor, offset=ap.offset, ap=list(ap.ap) + [[0, n]])


def build_nc(w: float) -> bass.Bass:
    nc = bacc.Bacc()

    # q/k/v rows quantized to int8 with a per-row bf16 scale packed in
    # the trailing 2 bytes (one third the upload bytes of f32).
    qkv_d = nc.dram_tensor("qkv", [3, PPC, L, 130], INT8, kind="ExternalInput")
    wh_d = nc.dram_tensor("wh", [PPC, 128, F], BF16, kind="ExternalInput")
    # output rows also ship as int8 + trailing per-row bf16 scale
    out_d = nc.dram_tensor("out", [PPC, NCH, 128, 130], INT8,
                           kind="ExternalOutput")

    with TileContext(nc) as tc:
        with (
            tc.tile_pool(name="sb", bufs=1) as sb,
            tc.tile_pool(name="small", bufs=2) as small,
            tc.tile_pool(name="const", bufs=1) as const,
            tc.tile_pool(name="grp", bufs=3) as grp,
            tc.tile_pool(name="pA", bufs=1, space="PSUM") as pA,
            tc.tile_pool(name="pSO", bufs=1, space="PSUM") as pSO,
            tc.tile_pool(name="pLQ", bufs=1, space="PSUM") as pLQ,
            tc.tile_pool(name="pST", bufs=2, space="PSUM") as pST,
        ):
            # identity matrix for PE-based 128x128 transposes
            ii = const.tile([128, 128], mybir.dt.int16, tag="ii")
            nc.gpsimd.iota(ii, pattern=[[1, 128]], channel_multiplier=-1)
            ident = const.tile([128, 128], BF16, tag="ident")
            nc.gpsimd.tensor_scalar(
                out=ident, in0=ii, scalar1=0, scalar2=None, op0=ALU.is_equal)

            for i in range(PPC):
                # ---- load pair inputs (int8 rows + trailing scale) ----
                qai = sb.tile([128, NCH, 130], INT8, tag="qai")
                nc.sync.dma_start(
                    out=qai,
                    in_=qkv_d[0][i].rearrange("(c p) k -> p c k", p=128))
                kai = sb.tile([128, NCH, 130], INT8, tag="kai")
                nc.sync.dma_start(
                    out=kai,
                    in_=qkv_d[1][i].rearrange("(c p) k -> p c k", p=128))
                vai = sb.tile([128, NCH, 130], INT8, tag="vai")
                nc.sync.dma_start(
                    out=vai,
                    in_=qkv_d[2][i].rearrange("(c p) k -> p c k", p=128))
                whs = small.tile([128, F], BF16, tag="wh")
                nc.sync.dma_start(out=whs, in_=wh_d[i])

                # dequantize v into [v|1] layout
                va = sb.tile([128, NCH, 130], BF16, tag="va")
                nc.vector.memset(va[:, :, 128:129], 1.0)
                vsc = vai[:, :, 128:130].bitcast(BF16)
                nc.vector.scalar_tensor_tensor(
                    va[:, :, 0:128], vai[:, :, 0:128], 1.0,
                    _bcast_last(vsc[:, :, 0], 128),
                    op0=ALU.mult, op1=ALU.mult)

                # dequantize q/k (natural layout), then transpose chunks
                # on the PE array to build qt/kt (D-major)
                qt = sb.tile([128, L], BF16, tag="qt")
                kt = sb.tile([128, L], BF16, tag="kt")
                for ai, nt in ((qai, qt), (kai, kt)):
                    nb = sb.tile([128, NCH, 128], BF16, tag="nb")
                    asc = ai[:, :, 128:130].bitcast(BF16)
                    nc.vector.scalar_tensor_tensor(
                        nb, ai[:, :, 0:128], 1.0,
                        _bcast_last(asc[:, :, 0], 128),
                        op0=ALU.mult, op1=ALU.mult)
                    for c4 in range(8):
                        pT = pA.tile([128, 512], BF16, tag="mmT")
                        for cc in range(4):
                            c = c4 * 4 + cc
                            nc.tensor.transpose(
                                pT[:, cc * 128:(cc + 1) * 128], nb[:, c, :],
                                ident)
                        nc.scalar.copy(nt[:, c4 * 512:(c4 + 1) * 512], pT)

                expq = sb.tile([128, L], BF16, tag="expq")
                expk = sb.tile([128, NCH, 128], BF16, tag="expk")
                phik = sb.tile([128, NCH, 128], BF16, tag="phik")
                outst = sb.tile([128, NCH, D], F16, tag="outst")

                # ---- phase A: q features (f-major, unnormalized) ----
                for j in range(8):
                    pu = pA.tile([128, 512], F32, tag="mm")
                    nc.tensor.matmul(
                        pu[0:64, :], lhsT=whs, rhs=qt[:, j * 512:(j + 1) * 512],
                        start=True, stop=True,
                    )
                    nc.scalar.activation(
                        expq[0:64, j * 512:(j + 1) * 512], pu[0:64, :], ACTF.Exp)
                    nc.scalar.activation(
                        expq[64:128, j * 512:(j + 1) * 512], pu[0:64, :], ACTF.Exp,
                        scale=-1.0)

                # ---- phase A: k features (natural layout) ----
                for jj in range(4):
                    pk = pA.tile([128, 512], F32, tag="mm")
                    for c8 in range(8):
                        c = jj * 8 + c8
                        nc.tensor.matmul(
                            pk[:, c8 * 64:(c8 + 1) * 64],
                            lhsT=kt[:, c * 128:(c + 1) * 128], rhs=whs,
                            start=True, stop=True,
                        )
                    pk3 = pk.rearrange("p (c f) -> p c f", f=64)
                    nc.scalar.activation(
                        expk[:, jj * 8:(jj + 1) * 8, 0:64], pk3, ACTF.Exp)
                    nc.scalar.activation(
                        expk[:, jj * 8:(jj + 1) * 8, 64:128], pk3, ACTF.Exp,
                        scale=-1.0)

                # normalize phi_k rows (per 64-feature half)
                sums = small.tile([128, NCH, 2], F32, tag="sums")
                nc.vector.tensor_reduce(
                    sums, expk.rearrange("p c (t f) -> p c t f", f=64),
                    axis=AX.X, op=ALU.add)
                recs = small.tile([128, NCH, 2], F32, tag="recs")
                nc.vector.reciprocal(recs, sums)
                for c in range(NCH):
                    for t in range(2):
                        nc.vector.tensor_scalar_mul(
                            phik[:, c, t * 64:(t + 1) * 64],
                            expk[:, c, t * 64:(t + 1) * 64],
                            recs[:, c, t:t + 1])

                # ---- phase B: block scan ----
                state = small.tile([128, 130], BF16, tag="state")
                nc.vector.memset(state[:, 0:129], 0.0)
                nc.vector.memset(state[:, 129:130], 1.0)
                sps_t = pST.tile([128, 512], F32, tag="st")
                sps = sps_t[:, 0:129]

                for g in range(NGRP):
                    c0, c1 = g * 128, (g + 1) * 128
                    # block-pair scores^T and exp
                    psc = pA.tile([128, 512], F32, tag="mm")
                    nc.tensor.matmul(
                        psc[:, 0:128], lhsT=kt[:, c0:c1], rhs=qt[:, c0:c1],
                        start=True, stop=True)
                    sst = grp.tile([128, 128], BF16, tag="sst")
                    nc.scalar.activation(sst, psc[:, 0:128], ACTF.Exp, scale=SCALING)

                    pso_t = pSO.tile([128, 512], F32, tag="so")
                    pso = pso_t[:, 0:129]
                    plq1_t = pLQ.tile([128, 512], F32, tag="lq1")
                    plq1 = plq1_t[:, 0:130]
                    plq2_t = pLQ.tile([128, 512], F32, tag="lq2")
                    plq2 = plq2_t[:, 0:130]

                    for h in range(2):  # even / odd block in the chunk
                        r0, r1 = h * 64, h * 64 + 64
                        # in-block softmax numerator @ [v|1]
                        nc.tensor.matmul(
                            pso[r0:r1, :], lhsT=sst[r0:r1, r0:r1],
                            rhs=va[r0:r1, g, 0:129],
                            start=True, stop=True, tile_position=(r0, r0))
                        # linear attention vs state (E and R halves)
                        nc.tensor.matmul(
                            plq1[r0:r1, 0:130],
                            lhsT=expq[0:64, c0 + h * 64: c0 + h * 64 + 64],
                            rhs=state[0:64, :],
                            start=True, stop=True, tile_position=(0, r0))
                        nc.tensor.matmul(
                            plq2[r0:r1, 0:130],
                            lhsT=expq[64:128, c0 + h * 64: c0 + h * 64 + 64],
                            rhs=state[64:128, :],
                            start=True, stop=True, tile_position=(64, r0))
                        # state update S += phi_k^T [v|1]
                        nc.tensor.matmul(
                            sps, lhsT=phik[r0:r1, g, :], rhs=va[r0:r1, g, 0:129],
                            start=(g == 0 and h == 0),
                            stop=(g == NGRP - 1 and h == 1),
                            skip_group_check=True,
                            tile_position=(r0, 0))
                        # refresh SBUF state copy for the next block
                        if not (g == NGRP - 1 and h == 1):
                            nc.scalar.copy(state[:, 0:129], sps)

                    # ---- assembly for the two blocks of this chunk ----
                    rs = grp.tile([128, 6], F32, tag="rs")
                    den = grp.tile([128, 2], F32, tag="den")
                    sc = grp.tile([128, 5], F32, tag="sc")
                    soev = grp.tile([128, 129], F32, tag="soev")
                    nc.scalar.copy(soev, pso)
                    lqev = grp.tile([128, 260], F32, tag="lqev")
                    nc.scalar.copy(lqev[:, 0:130], plq1)
                    nc.scalar.copy(lqev[:, 130:260], plq2)
                    nc.scalar.copy(sc[:, 0:1], soev[:, 128:129])
                    nc.scalar.copy(sc[:, 1:3], lqev[:, 128:130])
                    nc.scalar.copy(sc[:, 3:5], lqev[:, 258:260])
                    nc.vector.reciprocal(rs[:, 0:1], sc[:, 0:1])
                    nc.vector.reciprocal(rs[:, 1:2], sc[:, 2:3])
                    nc.vector.reciprocal(rs[:, 2:3], sc[:, 4:5])
                    nc.vector.tensor_scalar_mul(den[:, 0:1], sc[:, 1:2],
                                                rs[:, 1:2])
                    nc.vector.scalar_tensor_tensor(
                        den[:, 1:2], sc[:, 3:4], rs[:, 2:3], den[:, 0:1],
                        op0=ALU.mult, op1=ALU.add)
                    nc.vector.tensor_scalar_max(den[:, 0:1], den[:, 1:2], EPS)
                    nc.vector.reciprocal(rs[:, 3:4], den[:, 0:1])
                    nc.vector.tensor_scalar_mul(rs[:, 4:5], rs[:, 3:4], 1.0 - w)
                    nc.vector.tensor_scalar_mul(rs[:, 5:6], rs[:, 0:1], w)

                    t2 = grp.tile([128, 128], F32, tag="t2")
                    nc.vector.tensor_scalar_mul(t2, lqev[:, 0:128], rs[:, 1:2])
                    lin = grp.tile([128, 128], F32, tag="lin")
                    nc.vector.scalar_tensor_tensor(
                        lin, lqev[:, 130:258], rs[:, 2:3], t2,
                        op0=ALU.mult, op1=ALU.add)
                    sofl = grp.tile([128, 128], F32, tag="sofl")
                    nc.vector.tensor_scalar_mul(sofl, soev[:, 0:128], rs[:, 5:6])
                    nc.vector.scalar_tensor_tensor(
                        outst[:, g, :], lin, rs[:, 4:5], sofl,
                        op0=ALU.mult, op1=ALU.add)

                # quantize output rows to int8 with per-row bf16 scale
                # (abs-max via square -> max -> sqrt; walrus lacks abs_max)
                sqt = sb.tile([128, NCH, 128], BF16, tag="sqt")
                nc.vector.scalar_tensor_tensor(
                    sqt, outst, 1.0, outst, op0=ALU.mult, op1=ALU.mult)
                mx = small.tile([128, NCH, 1], F32, tag="mx")
                nc.vector.tensor_reduce(mx, sqt, axis=AX.X, op=ALU.max)
                oscf = small.tile([128, NCH, 1], F32, tag="oscf")
                nc.scalar.activation(
                    oscf, mx, ACTF.Sqrt, scale=1.0 / (126.0 * 126.0))
                osc = small.tile([128, NCH, 1], BF16, tag="osc")
                nc.vector.tensor_scalar_add(osc, oscf, 1e-30)
                oinv = small.tile([128, NCH, 1], F32, tag="oinv")
                nc.vector.reciprocal(oinv, osc)
                outq = sb.tile([128, NCH, 130], INT8, tag="outq")
                nc.vector.scalar_tensor_tensor(
                    outq[:, :, 0:128], outst, 1.0,
                    _bcast_last(oinv[:, :, 0], 128),
                    op0=ALU.mult, op1=ALU.mult)
                nc.scalar.copy(outq[:, :, 128:130].bitcast(BF16), osc)
                nc.sync.dma_start(out=out_d[i].rearrange("c p e -> p c e"),
                                  in_=outq)

    nc.compile()
    return nc


def _collect_io(nc):
    partition_name = (
        nc.partition_id_tensor.name if nc.partition_id_tensor else None)
    in_names, out_names, out_avals = [], [], []
    import jax
    for alloc in nc.m.functions[0].allocations:
        if not isinstance(alloc, mybir.MemoryLocationSet):
            continue
        name = alloc.memorylocations[0].name
        if alloc.kind == "ExternalInput":
            if name != partition_name:
                in_names.append(name)
        elif alloc.kind == "ExternalOutput":
            out_names.append(name)
            out_avals.append(jax.core.ShapedArray(
                tuple(alloc.tensor_shape), mybir.dt.np(alloc.dtype)))
    assert in_names == ["qkv", "wh"], in_names
    assert out_names == ["out"], out_names
    return partition_name, in_names, out_names, out_avals


def _make_body(nc, partition_name, in_names, out_names, out_avals):
    from concourse.bass2jax import _bass_exec_p, partition_id_tensor

    all_in_names = list(in_names) + list(out_names)
    if partition_name is not None:
        all_in_names.append(partition_name)

    def _body(*args):
        operands = list(args)
        if partition_name is not None:
            operands.append(partition_id_tensor())
        outs = _bass_exec_p.bind(
            *operands,
            out_avals=tuple(out_avals),
            in_names=tuple(all_in_names),
            out_names=tuple(out_names),
            lowering_input_output_aliases=(),
            sim_require_finite=True,
            sim_require_nnan=True,
            nc=nc,
        )
        return tuple(outs)

    return _body


def _pack_pair_into(dst, qr, kr, vr):
    """Quantize one pair's q/k/v rows into dst int8 [3, L, 130]."""
    for t, x in enumerate((qr, kr, vr)):
        # /126 leaves headroom for the bf16 scale rounding down, so the
        # rinted values can never exceed 127 and no clip pass is needed
        s = (np.abs(x).max(-1) / 126.0 + 1e-30).astype(BFDT)
        inv = 1.0 / s.astype(np.float32)
        tq = x * inv[:, None]
        np.rint(tq, out=tq)
        d = dst[t]
        d[:, 0:128] = tq
        d[:, 128:130] = s.view(np.int8).reshape(L, 2)


# ----------------------------------------------------------------------
# worker process
# ----------------------------------------------------------------------

def _worker_main(widx, in_name, out_name, wh_name):
    from multiprocessing import shared_memory
    from concurrent.futures import ThreadPoolExecutor

    # protocol writes go to the original stdout; the runtime's C-level
    # stdout is redirected to stderr so it cannot corrupt the protocol
    proto_fd = os.dup(1)
    os.dup2(2, 1)
    proto = os.fdopen(proto_fd, "w", buffering=1)

    # track=False: the attaching process must not unlink the segments
    # when it exits (py3.13 resource-tracker behavior)
    shm_in = shared_memory.SharedMemory(name=in_name, track=False)
    shm_out = shared_memory.SharedMemory(name=out_name, track=False)
    shm_wh = shared_memory.SharedMemory(name=wh_name, track=False)
    in_view = np.ndarray((PAIRS, 3, L, 130), np.int8, buffer=shm_in.buf)
    out_view = np.ndarray((PAIRS, NCH, 128, 128), np.float32,
                          buffer=shm_out.buf)
    wh_view = np.ndarray((H, 128, F), np.float32, buffer=shm_wh.buf)

    state = {}
    pool = ThreadPoolExecutor(max_workers=2)
    lock = threading.Lock()

    def _say(msg):
        with lock:
            proto.write(msg + "\n")
            proto.flush()

    def _init(wv):
        import jax
        from concourse.bass2jax import install_neuronx_cc_hook
        if state.get("w") == wv:
            return
        install_neuronx_cc_hook()
        nc = build_nc(wv)
        pn, in_names, out_names, out_avals = _collect_io(nc)
        body = _make_body(nc, pn, in_names, out_names, out_avals)
        fn = jax.jit(body, keep_unused=True)
        dev = jax.devices()[widx]
        zeros = [jax.device_put(np.zeros(a.shape, a.dtype), dev)
                 for a in out_avals]
        state.update(w=wv, fn=fn, dev=dev, zeros=zeros, jax=jax, whc={})
        # warm: trace+compile+NEFF load now, not on the first real pair
        dummy_q = jax.device_put(
            np.zeros((3, PPC, L, 130), np.int8), dev)
        dummy_w = jax.device_put(
            np.zeros((PPC, 128, F), BFDT), dev)
        o = fn(dummy_q, dummy_w, *zeros)[0]
        np.asarray(o)

    def _fetch(o, p):
        part = np.asarray(o)[0]                    # [NCH,128,130] int8
        sc = np.ascontiguousarray(part[:, :, 128:130]).view(
            BFDT).astype(np.float32)               # [NCH,128,1]
        np.multiply(part[:, :, 0:128], sc, out=out_view[p])
        _say(f"done {p}")

    def _pair(p):
        jax = state["jax"]
        dev = state["dev"]
        qkv_dev = jax.device_put(in_view[p].reshape(3, PPC, L, 130), dev)
        h = p % H
        whc = state["whc"]
        if state.get("whseq") != state["seq"]:
            whc.clear()
            state["whseq"] = state["seq"]
        if h not in whc:
            whc[h] = jax.device_put(
                np.ascontiguousarray(wh_view[h][None]).astype(BFDT), dev)
        o = state["fn"](qkv_dev, whc[h], *state["zeros"])[0]
        pool.submit(_fetch, o, p)

    _say("ready")
    try:
        for line in sys.stdin:
            parts = line.split()
            if not parts:
                continue
            if parts[0] == "init":
                try:
                    _init(float(parts[1]))
                    _say("inited")
                except Exception:
                    import traceback
                    traceback.print_exc(file=sys.stderr)
                    _say("initfail")
            elif parts[0] == "call":
                state["seq"] = int(parts[1])
            elif parts[0] == "pair":
                _pair(int(parts[1]))
            elif parts[0] == "quit":
                break
    finally:
        pool.shutdown(wait=False)


# ----------------------------------------------------------------------
# parent-side multiprocess driver
# ----------------------------------------------------------------------

_MP = {"procs": None}


class _WorkerHandle:
    def __init__(self, proc):
        self.proc = proc
        self.ready = threading.Event()
        self.inited = threading.Event()
        self.initfail = False
        self.dead = False
        self.dones = 0
        self.cond = threading.Condition()
        t = threading.Thread(target=self._reader, daemon=True)
        t.start()

    def _reader(self):
        for line in self.proc.stdout:
            s = line.strip()
            if s == "ready":
                self.ready.set()
            elif s == "inited":
                self.inited.set()
            elif s == "initfail":
                self.initfail = True
                self.inited.set()
            elif s.startswith("done "):
                with self.cond:
                    self.dones += 1
                    self.cond.notify()
        self.dead = True
        self.ready.set()
        self.inited.set()
        with self.cond:
            self.cond.notify()

    def send(self, msg):
        self.proc.stdin.write(msg + "\n")
        self.proc.stdin.flush()


def _mp_ensure():
    import subprocess
    from multiprocessing import shared_memory

    if _MP["procs"] is not None:
        return
    tag = f"bsk{os.getpid()}"
    shm_in = shared_memory.SharedMemory(
        create=True, size=PAIRS * 3 * L * 130, name=f"{tag}i")
    shm_out = shared_memory.SharedMemory(
        create=True, size=PAIRS * L * D * 4, name=f"{tag}o")
    shm_wh = shared_memory.SharedMemory(
        create=True, size=H * 128 * F * 4, name=f"{tag}w")
    procs = []
    for wi in range(NW):
        code = (
            "import sys; sys.path.insert(0, %r); import kernel; "
            "kernel._worker_main(%d, %r, %r, %r)"
            % (_KDIR, wi, f"{tag}i", f"{tag}o", f"{tag}w"))
        try:
            errdst = sys.stderr if sys.stderr.fileno() >= 0 else None
        except Exception:
            errdst = subprocess.DEVNULL
        p = subprocess.Popen(
            [sys.executable, "-u", "-c", code],
            stdin=subprocess.PIPE, stdout=subprocess.PIPE,
            stderr=errdst, text=True)
        procs.append(_WorkerHandle(p))
    _MP.update(procs=procs, shm_in=shm_in, shm_out=shm_out, shm_wh=shm_wh,
               in_view=np.ndarray((PAIRS, 3, L, 130), np.int8,
                                  buffer=shm_in.buf),
               out_view=np.ndarray((PAIRS, NCH, 128, 128), np.float32,
                                   buffer=shm_out.buf),
               wh_view=np.ndarray((H, 128, F), np.float32,
                                  buffer=shm_wh.buf),
               w=None, seq=0)


def _mp_run(q, k, v, wts, w):
    _mp_ensure()
    procs = _MP["procs"]
    for h in procs:
        if not h.ready.wait(timeout=300) or h.dead:
            raise RuntimeError("worker failed to start")
    if _MP["w"] != w:
        for h in procs:
            h.send(f"init {w!r}")
        for h in procs:
            if not h.inited.wait(timeout=900) or h.initfail or h.dead:
                raise RuntimeError("worker init failed")
            h.inited.clear()
        _MP["w"] = w
    _MP["seq"] += 1
    seq = _MP["seq"]
    _MP["wh_view"][:] = wts
    for h in procs:
        h.send(f"call {seq}")
    in_view = _MP["in_view"]
    qf = q.reshape(PAIRS, L, D)
    kf = k.reshape(PAIRS, L, D)
    vf = v.reshape(PAIRS, L, D)
    for p in range(PAIRS):
        _pack_pair_into(in_view[p], qf[p], kf[p], vf[p])
        procs[p % NW].send(f"pair {p}")
    deadline = time.time() + 300
    per_w = PAIRS // NW
    for h in procs:
        with h.cond:
            while h.dones < per_w and not h.dead:
                if not h.cond.wait(timeout=max(0.1, deadline - time.time())):
                    raise RuntimeError("worker timed out")
            if h.dead and h.dones < per_w:
                raise RuntimeError("worker died")
            h.dones = 0
    return _MP["out_view"].reshape(B, H, L, D).copy()


# ----------------------------------------------------------------------
# single-process fallback (8-core shard_map), and host reference
# ----------------------------------------------------------------------

_STATE = {}


def _build_state(w: float):
    import jax
    from jax.sharding import Mesh, PartitionSpec, NamedSharding
    from jax.experimental.shard_map import shard_map
    from concourse.bass2jax import install_neuronx_cc_hook

    nc = build_nc(w)
    install_neuronx_cc_hook()
    pn, in_names, out_names, out_avals = _collect_io(nc)
    body = _make_body(nc, pn, in_names, out_names, out_avals)
    n_params = len(in_names)
    n_outs = len(out_names)

    devices = jax.devices()[:NCORES]
    mesh = Mesh(np.asarray(devices), ("core",))
    spec = NamedSharding(mesh, PartitionSpec("core"))
    fn = jax.jit(
        shard_map(body, mesh=mesh,
                  in_specs=(PartitionSpec("core"),) * (n_params + n_outs),
                  out_specs=(PartitionSpec("core"),) * n_outs,
                  check_rep=False),
        keep_unused=True,
    )
    zeros = [
        jax.device_put(
            np.zeros((NCORES * a.shape[0], *a.shape[1:]), a.dtype), spec)
        for a in out_avals
    ]
    return {"fn": fn, "spec": spec, "zeros": zeros}


def _get_state(w: float):
    key = round(w, 10)
    if key not in _STATE:
        _STATE[key] = _build_state(w)
    return _STATE[key]


def _pack_qkv(q, k, v, j):
    sl = slice(j * RPC, (j + 1) * RPC)
    pack = np.empty((NCORES, 3, PPC, L, 130), dtype=np.int8)
    for t, x in enumerate((q, k, v)):
        xx = x[sl]
        s = (np.abs(xx).max(-1) / 126.0 + 1e-30).astype(BFDT)
        inv = 1.0 / s.astype(np.float32)
        tq = xx * inv[..., None]
        np.rint(tq, out=tq)
        dst = pack[:, t]
        dst[:, :, :, 0:128] = tq.reshape(NCORES, PPC, L, D)
        dst[:, :, :, 128:130] = s.view(np.int8).reshape(NCORES, PPC, L, 2)
    return pack.reshape(NCORES * 3, PPC, L, 130)


def _run_device(q, k, v, wts, w):
    import math

    import jax
    from concurrent.futures import ThreadPoolExecutor

    st = _get_state(w)
    qf = q.reshape(PAIRS, L, D)
    kf = k.reshape(PAIRS, L, D)
    vf = v.reshape(PAIRS, L, D)
    period = H // math.gcd(H, RPC)
    wh_devs = None

    fetches = []
    res = np.empty((PAIRS, NCH, 128, D), dtype=np.float32)

    def _fetch_into(out_arr, j):
        part = np.asarray(out_arr)            # [RPC, NCH, 128, 130] int8
        sc = np.ascontiguousarray(part[..., 128:130]).view(
            BFDT).astype(np.float32)
        sl = slice(j * RPC, (j + 1) * RPC)
        res[sl] = part[..., 0:128]
        res[sl] *= sc

    with ThreadPoolExecutor(max_workers=4) as ex:
        for j in range(NCALL):
            qkv_dev = jax.device_put(_pack_qkv(qf, kf, vf, j), st["spec"])
            if wh_devs is None:
                wh_devs = [
                    jax.device_put(
                        np.asarray(wts[(jj * RPC + np.arange(RPC)) % H],
                                   dtype=BFDT),
                        st["spec"])
                    for jj in range(period)
                ]
            o = st["fn"](qkv_dev, wh_devs[j % period], *st["zeros"])[0]
            fetches.append(ex.submit(_fetch_into, o, j))
        for f in fetches:
            f.result()

    return res.reshape(B, H, L, D)


def kernel(query_states, key_states, value_states, hedgehog_weights, alpha):
    q = np.asarray(query_states, dtype=np.float32)
    k = np.asarray(key_states, dtype=np.float32)
    v = np.asarray(value_states, dtype=np.float32)
    wts = np.asarray(hedgehog_weights, dtype=np.float32)
    a = float(np.asarray(alpha))
    w = float(1.0 / (1.0 + np.exp(-a)))

    if _MP.get("ok", True):
        try:
            return _mp_run(q, k, v, wts, w)
        except Exception:
            import traceback
            traceback.print_exc(file=sys.stderr)
            _MP["ok"] = False
    try:
        return _run_device(q, k, v, wts, w)
    except Exception:
        import traceback
        traceback.print_exc(file=sys.stderr)
        return _host_reference(q, k, v, wts, w)


def _host_reference(q, k, v, wts, w):
    # Last-resort fallback so a transient device failure still returns
    # a correct result; mirrors the block-scan math in fp32 numpy.
    out = np.empty((B, H, L, D), dtype=np.float32)
    for b in range(B):
        for h in range(H):
            u = q[b, h].reshape(NBLK, SBLK, D) @ wts[h]
            pq = np.concatenate([_sm(u), _sm(-u)], -1)
            uk = k[b, h].reshape(NBLK, SBLK, D) @ wts[h]
            pk = np.concatenate([_sm(uk), _sm(-uk)], -1)
            vb = v[b, h].reshape(NBLK, SBLK, D)
            qb = q[b, h].reshape(NBLK, SBLK, D)
            kb = k[b, h].reshape(NBLK, SBLK, D)
            S = np.zeros((2 * F, D), np.float32)
            Z = np.zeros((2 * F,), np.float32)
            for n in range(NBLK):
                den = np.maximum(pq[n] @ Z, EPS)
                lin = (pq[n] @ S) / den[:, None]
                S = S + pk[n].T @ vb[n]
                Z = Z + pk[n].sum(0)
                sc = qb[n] @ kb[n].T * SCALING
                p = _sm(sc)
                out[b, h, n * SBLK:(n + 1) * SBLK] = (
                    w * (p @ vb[n]) + (1 - w) * lin)
    return out


def _sm(x):
    e = np.exp(x - x.max(-1, keepdims=True))
    return e / e.sum(-1, keepdims=True)


# revision 15
# speedup vs baseline: 1.0107x; 1.0107x over previous
"""Trainium2 Bass kernel for BlockSoftmaxLinearHybrid.

The warm-call wall time is dominated by the axon tunnel, whose
bandwidth is limited PER CLIENT CONNECTION (~35MB/s each way today,
but N independent client processes scale to ~N x that).  So the host
side runs 8 persistent worker processes, each with its own axon client
session and one NeuronCore, each handling 4 of the 32 (b,h) pairs:

  * the parent quantizes q/k/v rows to int8 (+ per-row bf16 scale in
    the trailing 2 bytes) directly into shared memory, round-robin one
    pair at a time so all 8 connections go busy almost immediately;
  * each worker uploads its pairs as they arrive, runs the Bass kernel
    (one exec per pair), downloads the int8 output and dequantizes it
    into a shared f32 output buffer;
  * emulated end-to-end rel-err with q/k/v and the output all at int8
    is 0.0152 vs the 0.02 gate.

If worker spawn/init fails, falls back to a single-process 8-core
shard_map path, then to a host numpy reference.

Device kernel per (b,h) pair:
  phase A: dequantize q/k, transpose both on the PE array to D-major;
           u_q^T = W^T Q^T (f-major), EXPQ=[exp(u);exp(-u)] unnormalized
           (normalization recovered via ones-column in the state matmul);
           u_k in natural layout, exp'd and row-normalized -> phi_k.
  phase B: per 64-row block scan: block-local softmax attention
           (scores^T -> exp -> @[v|1]) + linear attention vs the running
           [S|Z] state accumulated in PSUM, blended with w=sigmoid(alpha).
"""

import os
import sys
import threading
import time

import numpy as np

if "/opt/trn_rl_repo" not in sys.path:
    sys.path.insert(0, "/opt/trn_rl_repo")
_KDIR = os.path.dirname(os.path.abspath(__file__))
if _KDIR not in sys.path:
    sys.path.insert(0, _KDIR)

import ml_dtypes

import concourse.bass as bass
import concourse.bacc as bacc
import concourse.mybir as mybir
from concourse.tile import TileContext

B, H, L, D = 2, 16, 4096, 128
F = 64          # feature dim; phi dim is 2F = 128
SBLK = 64       # block size
NBLK = L // SBLK            # 64 blocks
NCH = L // 128              # 32 chunks (2 blocks each)
EPS = 1e-6
SCALING = D ** -0.5
NCORES = 8
PAIRS = B * H               # 32 (b,h) pairs
NCALL = 4                   # fallback path: pipeline chunks per call
RPC = PAIRS // NCALL        # fallback: pairs per chunk
PPC = RPC // NCORES         # pairs per core per exec (= 1)
NGRP = NCH                  # phase-B group count
NW = 8                      # worker processes (one per core)

BF16 = mybir.dt.bfloat16
F16 = mybir.dt.float16
F32 = mybir.dt.float32
INT8 = mybir.dt.int8
AX = mybir.AxisListType
ALU = mybir.AluOpType
ACTF = mybir.ActivationFunctionType
BFDT = ml_dtypes.bfloat16


def _bcast_last(ap, n):
    """Append a stride-0 dim of size n to an AP (free-dim broadcast)."""
    return bass.AP(tensor=ap.tensor, offset=ap.offset, ap=list(ap.ap) + [[0, n]])


def build_nc(w: float) -> bass.Bass:
    nc = bacc.Bacc()

    # q/k/v rows quantized to int8 with a per-row bf16 scale packed in
    # the trailing 2 bytes (one third the upload bytes of f32).
    qkv_d = nc.dram_tensor("qkv", [3, PPC, L, 130], INT8, kind="ExternalInput")
    wh_d = nc.dram_tensor("wh", [PPC, 128, F], BF16, kind="ExternalInput")
    # output rows also ship as int8 + trailing per-row bf16 scale
    out_d = nc.dram_tensor("out", [PPC, NCH, 128, 130], INT8,
                           kind="ExternalOutput")

    with TileContext(nc) as tc:
        with (
            tc.tile_pool(name="sb", bufs=1) as sb,
            tc.tile_pool(name="small", bufs=2) as small,
            tc.tile_pool(name="const", bufs=1) as const,
            tc.tile_pool(name="grp", bufs=3) as grp,
            tc.tile_pool(name="pA", bufs=1, space="PSUM") as pA,
            tc.tile_pool(name="pSO", bufs=1, space="PSUM") as pSO,
            tc.tile_pool(name="pLQ", bufs=1, space="PSUM") as pLQ,
            tc.tile_pool(name="pST", bufs=2, space="PSUM") as pST,
        ):
            # identity matrix for PE-based 128x128 transposes
            ii = const.tile([128, 128], mybir.dt.int16, tag="ii")
            nc.gpsimd.iota(ii, pattern=[[1, 128]], channel_multiplier=-1)
            ident = const.tile([128, 128], BF16, tag="ident")
            nc.gpsimd.tensor_scalar(
                out=ident, in0=ii, scalar1=0, scalar2=None, op0=ALU.is_equal)

            for i in range(PPC):
                # ---- load pair inputs (int8 rows + trailing scale) ----
                qai = sb.tile([128, NCH, 130], INT8, tag="qai")
                nc.sync.dma_start(
                    out=qai,
                    in_=qkv_d[0][i].rearrange("(c p) k -> p c k", p=128))
                kai = sb.tile([128, NCH, 130], INT8, tag="kai")
                nc.sync.dma_start(
                    out=kai,
                    in_=qkv_d[1][i].rearrange("(c p) k -> p c k", p=128))
                vai = sb.tile([128, NCH, 130], INT8, tag="vai")
                nc.sync.dma_start(
                    out=vai,
                    in_=qkv_d[2][i].rearrange("(c p) k -> p c k", p=128))
                whs = small.tile([128, F], BF16, tag="wh")
                nc.sync.dma_start(out=whs, in_=wh_d[i])

                # dequantize v into [v|1] layout
                va = sb.tile([128, NCH, 130], BF16, tag="va")
                nc.vector.memset(va[:, :, 128:129], 1.0)
                vsc = vai[:, :, 128:130].bitcast(BF16)
                nc.vector.scalar_tensor_tensor(
                    va[:, :, 0:128], vai[:, :, 0:128], 1.0,
                    _bcast_last(vsc[:, :, 0], 128),
                    op0=ALU.mult, op1=ALU.mult)

                # dequantize q/k (natural layout), then transpose chunks
                # on the PE array to build qt/kt (D-major)
                qt = sb.tile([128, L], BF16, tag="qt")
                kt = sb.tile([128, L], BF16, tag="kt")
                for ai, nt in ((qai, qt), (kai, kt)):
                    nb = sb.tile([128, NCH, 128], BF16, tag="nb")
                    asc = ai[:, :, 128:130].bitcast(BF16)
                    nc.vector.scalar_tensor_tensor(
                        nb, ai[:, :, 0:128], 1.0,
                        _bcast_last(asc[:, :, 0], 128),
                        op0=ALU.mult, op1=ALU.mult)
                    for c4 in range(8):
                        pT = pA.tile([128, 512], BF16, tag="mmT")
                        for cc in range(4):
                            c = c4 * 4 + cc
                            nc.tensor.transpose(
                                pT[:, cc * 128:(cc + 1) * 128], nb[:, c, :],
                                ident)
                        nc.scalar.copy(nt[:, c4 * 512:(c4 + 1) * 512], pT)

                expq = sb.tile([128, L], BF16, tag="expq")
                expk = sb.tile([128, NCH, 128], BF16, tag="expk")
                phik = sb.tile([128, NCH, 128], BF16, tag="phik")
                outst = sb.tile([128, NCH, D], F16, tag="outst")

                # ---- phase A: q features (f-major, unnormalized) ----
                for j in range(8):
                    pu = pA.tile([128, 512], F32, tag="mm")
                    nc.tensor.matmul(
                        pu[0:64, :], lhsT=whs, rhs=qt[:, j * 512:(j + 1) * 512],
                        start=True, stop=True,
                    )
                    nc.scalar.activation(
                        expq[0:64, j * 512:(j + 1) * 512], pu[0:64, :], ACTF.Exp)
                    nc.scalar.activation(
                        expq[64:128, j * 512:(j + 1) * 512], pu[0:64, :], ACTF.Exp,
                        scale=-1.0)

                # ---- phase A: k features (natural layout) ----
                for jj in range(4):
                    pk = pA.tile([128, 512], F32, tag="mm")
                    for c8 in range(8):
                        c = jj * 8 + c8
                        nc.tensor.matmul(
                            pk[:, c8 * 64:(c8 + 1) * 64],
                            lhsT=kt[:, c * 128:(c + 1) * 128], rhs=whs,
                            start=True, stop=True,
                        )
                    pk3 = pk.rearrange("p (c f) -> p c f", f=64)
                    nc.scalar.activation(
                        expk[:, jj * 8:(jj + 1) * 8, 0:64], pk3, ACTF.Exp)
                    nc.scalar.activation(
                        expk[:, jj * 8:(jj + 1) * 8, 64:128], pk3, ACTF.Exp,
                        scale=-1.0)

                # normalize phi_k rows (per 64-feature half)
                sums = small.tile([128, NCH, 2], F32, tag="sums")
                nc.vector.tensor_reduce(
                    sums, expk.rearrange("p c (t f) -> p c t f", f=64),
                    axis=AX.X, op=ALU.add)
                recs = small.tile([128, NCH, 2], F32, tag="recs")
                nc.vector.reciprocal(recs, sums)
                for c in range(NCH):
                    for t in range(2):
                        nc.vector.tensor_scalar_mul(
                            phik[:, c, t * 64:(t + 1) * 64],
                            expk[:, c, t * 64:(t + 1) * 64],
                            recs[:, c, t:t + 1])

                # ---- phase B: block scan ----
                state = small.tile([128, 130], BF16, tag="state")
                nc.vector.memset(state[:, 0:129], 0.0)
                nc.vector.memset(state[:, 129:130], 1.0)
                sps_t = pST.tile([128, 512], F32, tag="st")
                sps = sps_t[:, 0:129]

                for g in range(NGRP):
                    c0, c1 = g * 128, (g + 1) * 128
                    # block-pair scores^T and exp
                    psc = pA.tile([128, 512], F32, tag="mm")
                    nc.tensor.matmul(
                        psc[:, 0:128], lhsT=kt[:, c0:c1], rhs=qt[:, c0:c1],
                        start=True, stop=True)
                    sst = grp.tile([128, 128], BF16, tag="sst")
                    nc.scalar.activation(sst, psc[:, 0:128], ACTF.Exp, scale=SCALING)

                    pso_t = pSO.tile([128, 512], F32, tag="so")
                    pso = pso_t[:, 0:129]
                    plq1_t = pLQ.tile([128, 512], F32, tag="lq1")
                    plq1 = plq1_t[:, 0:130]
                    plq2_t = pLQ.tile([128, 512], F32, tag="lq2")
                    plq2 = plq2_t[:, 0:130]

                    for h in range(2):  # even / odd block in the chunk
                        r0, r1 = h * 64, h * 64 + 64
                        # in-block softmax numerator @ [v|1]
                        nc.tensor.matmul(
                            pso[r0:r1, :], lhsT=sst[r0:r1, r0:r1],
                            rhs=va[r0:r1, g, 0:129],
                            start=True, stop=True, tile_position=(r0, r0))
                        # linear attention vs state (E and R halves)
                        nc.tensor.matmul(
                            plq1[r0:r1, 0:130],
                            lhsT=expq[0:64, c0 + h * 64: c0 + h * 64 + 64],
                            rhs=state[0:64, :],
                            start=True, stop=True, tile_position=(0, r0))
                        nc.tensor.matmul(
                            plq2[r0:r1, 0:130],
                            lhsT=expq[64:128, c0 + h * 64: c0 + h * 64 + 64],
                            rhs=state[64:128, :],
                            start=True, stop=True, tile_position=(64, r0))
                        # state update S += phi_k^T [v|1]
                        nc.tensor.matmul(
                            sps, lhsT=phik[r0:r1, g, :], rhs=va[r0:r1, g, 0:129],
                            start=(g == 0 and h == 0),
                            stop=(g == NGRP - 1 and h == 1),
                            skip_group_check=True,
                            tile_position=(r0, 0))
                        # refresh SBUF state copy for the next block
                        if not (g == NGRP - 1 and h == 1):
                            nc.scalar.copy(state[:, 0:129], sps)

                    # ---- assembly for the two blocks of this chunk ----
                    rs = grp.tile([128, 6], F32, tag="rs")
                    den = grp.tile([128, 2], F32, tag="den")
                    sc = grp.tile([128, 5], F32, tag="sc")
                    soev = grp.tile([128, 129], F32, tag="soev")
                    nc.scalar.copy(soev, pso)
                    lqev = grp.tile([128, 260], F32, tag="lqev")
                    nc.scalar.copy(lqev[:, 0:130], plq1)
                    nc.scalar.copy(lqev[:, 130:260], plq2)
                    nc.scalar.copy(sc[:, 0:1], soev[:, 128:129])
                    nc.scalar.copy(sc[:, 1:3], lqev[:, 128:130])
                    nc.scalar.copy(sc[:, 3:5], lqev[:, 258:260])
                    nc.vector.reciprocal(rs[:, 0:1], sc[:, 0:1])
                    nc.vector.reciprocal(rs[:, 1:2], sc[:, 2:3])
                    nc.vector.reciprocal(rs[:, 2:3], sc[:, 4:5])
                    nc.vector.tensor_scalar_mul(den[:, 0:1], sc[:, 1:2],
                                                rs[:, 1:2])
                    nc.vector.scalar_tensor_tensor(
                        den[:, 1:2], sc[:, 3:4], rs[:, 2:3], den[:, 0:1],
                        op0=ALU.mult, op1=ALU.add)
                    nc.vector.tensor_scalar_max(den[:, 0:1], den[:, 1:2], EPS)
                    nc.vector.reciprocal(rs[:, 3:4], den[:, 0:1])
                    nc.vector.tensor_scalar_mul(rs[:, 4:5], rs[:, 3:4], 1.0 - w)
                    nc.vector.tensor_scalar_mul(rs[:, 5:6], rs[:, 0:1], w)

                    t2 = grp.tile([128, 128], F32, tag="t2")
                    nc.vector.tensor_scalar_mul(t2, lqev[:, 0:128], rs[:, 1:2])
                    lin = grp.tile([128, 128], F32, tag="lin")
                    nc.vector.scalar_tensor_tensor(
                        lin, lqev[:, 130:258], rs[:, 2:3], t2,
                        op0=ALU.mult, op1=ALU.add)
                    sofl = grp.tile([128, 128], F32, tag="sofl")
                    nc.vector.tensor_scalar_mul(sofl, soev[:, 0:128], rs[:, 5:6])
                    nc.vector.scalar_tensor_tensor(
                        outst[:, g, :], lin, rs[:, 4:5], sofl,
                        op0=ALU.mult, op1=ALU.add)

                # quantize output rows to int8 with per-row bf16 scale
                # (abs-max via square -> max -> sqrt; walrus lacks abs_max)
                sqt = sb.tile([128, NCH, 128], BF16, tag="sqt")
                nc.vector.scalar_tensor_tensor(
                    sqt, outst, 1.0, outst, op0=ALU.mult, op1=ALU.mult)
                mx = small.tile([128, NCH, 1], F32, tag="mx")
                nc.vector.tensor_reduce(mx, sqt, axis=AX.X, op=ALU.max)
                oscf = small.tile([128, NCH, 1], F32, tag="oscf")
                nc.scalar.activation(
                    oscf, mx, ACTF.Sqrt, scale=1.0 / (126.0 * 126.0))
                osc = small.tile([128, NCH, 1], BF16, tag="osc")
                nc.vector.tensor_scalar_add(osc, oscf, 1e-30)
                oinv = small.tile([128, NCH, 1], F32, tag="oinv")
                nc.vector.reciprocal(oinv, osc)
                outq = sb.tile([128, NCH, 130], INT8, tag="outq")
                nc.vector.scalar_tensor_tensor(
                    outq[:, :, 0:128], outst, 1.0,
                    _bcast_last(oinv[:, :, 0], 128),
                    op0=ALU.mult, op1=ALU.mult)
                nc.scalar.copy(outq[:, :, 128:130].bitcast(BF16), osc)
                nc.sync.dma_start(out=out_d[i].rearrange("c p e -> p c e"),
                                  in_=outq)

    nc.compile()
    return nc


def _collect_io(nc):
    partition_name = (
        nc.partition_id_tensor.name if nc.partition_id_tensor else None)
    in_names, out_names, out_avals = [], [], []
    import jax
    for alloc in nc.m.functions[0].allocations:
        if not isinstance(alloc, mybir.MemoryLocationSet):
            continue
        name = alloc.memorylocations[0].name
        if alloc.kind == "ExternalInput":
            if name != partition_name:
                in_names.append(name)
        elif alloc.kind == "ExternalOutput":
            out_names.append(name)
            out_avals.append(jax.core.ShapedArray(
                tuple(alloc.tensor_shape), mybir.dt.np(alloc.dtype)))
    assert in_names == ["qkv", "wh"], in_names
    assert out_names == ["out"], out_names
    return partition_name, in_names, out_names, out_avals


def _make_body(nc, partition_name, in_names, out_names, out_avals):
    from concourse.bass2jax import _bass_exec_p, partition_id_tensor

    all_in_names = list(in_names) + list(out_names)
    if partition_name is not None:
        all_in_names.append(partition_name)

    def _body(*args):
        operands = list(args)
        if partition_name is not None:
            operands.append(partition_id_tensor())
        outs = _bass_exec_p.bind(
            *operands,
            out_avals=tuple(out_avals),
            in_names=tuple(all_in_names),
            out_names=tuple(out_names),
            lowering_input_output_aliases=(),
            sim_require_finite=True,
            sim_require_nnan=True,
            nc=nc,
        )
        return tuple(outs)

    return _body


def _pack_pair_into(dst, qr, kr, vr):
    """Quantize one pair's q/k/v rows into dst int8 [3, L, 130]."""
    for t, x in enumerate((qr, kr, vr)):
        # /126 leaves headroom for the bf16 scale rounding down, so the
        # rinted values can never exceed 127 and no clip pass is needed
        s = (np.abs(x).max(-1) / 126.0 + 1e-30).astype(BFDT)
        inv = 1.0 / s.astype(np.float32)
        tq = x * inv[:, None]
        np.rint(tq, out=tq)
        d = dst[t]
        d[:, 0:128] = tq
        d[:, 128:130] = s.view(np.int8).reshape(L, 2)


# ----------------------------------------------------------------------
# worker process
# ----------------------------------------------------------------------

def _worker_main(widx, in_name, out_name, wh_name):
    from multiprocessing import shared_memory
    from concurrent.futures import ThreadPoolExecutor

    # protocol writes go to the original stdout; the runtime's C-level
    # stdout is redirected to stderr so it cannot corrupt the protocol
    proto_fd = os.dup(1)
    os.dup2(2, 1)
    proto = os.fdopen(proto_fd, "w", buffering=1)

    # track=False: the attaching process must not unlink the segments
    # when it exits (py3.13 resource-tracker behavior)
    shm_in = shared_memory.SharedMemory(name=in_name, track=False)
    shm_out = shared_memory.SharedMemory(name=out_name, track=False)
    shm_wh = shared_memory.SharedMemory(name=wh_name, track=False)
    in_view = np.ndarray((PAIRS, 3, L, 130), np.int8, buffer=shm_in.buf)
    out_view = np.ndarray((PAIRS, NCH, 128, 128), np.float32,
                          buffer=shm_out.buf)
    wh_view = np.ndarray((H, 128, F), np.float32, buffer=shm_wh.buf)

    state = {}
    pool = ThreadPoolExecutor(max_workers=2)
    lock = threading.Lock()

    def _say(msg):
        with lock:
            proto.write(msg + "\n")
            proto.flush()

    def _init(wv):
        t0 = time.time()

        def _lap(msg):
            print(f"[w{widx}] {msg} @{time.time() - t0:.1f}s",
                  file=sys.stderr, flush=True)

        import jax
        from concourse.bass2jax import install_neuronx_cc_hook
        if state.get("w") == wv:
            return
        _lap("jax imported")
        install_neuronx_cc_hook()
        nc = build_nc(wv)
        _lap("bass built")
        pn, in_names, out_names, out_avals = _collect_io(nc)
        body = _make_body(nc, pn, in_names, out_names, out_avals)
        fn = jax.jit(body, keep_unused=True)
        dev = jax.devices()[widx]
        _lap("devices up")
        zeros = [jax.device_put(np.zeros(a.shape, a.dtype), dev)
                 for a in out_avals]
        state.update(w=wv, fn=fn, dev=dev, zeros=zeros, jax=jax, whc={})
        # warm: trace+compile+NEFF load now, not on the first real pair
        dummy_q = jax.device_put(
            np.zeros((3, PPC, L, 130), np.int8), dev)
        dummy_w = jax.device_put(
            np.zeros((PPC, 128, F), BFDT), dev)
        o = fn(dummy_q, dummy_w, *zeros)[0]
        _lap("jit compiled+dispatched")
        np.asarray(o)
        _lap("warm exec done")

    def _fetch(o, p):
        part = np.asarray(o)[0]                    # [NCH,128,130] int8
        sc = np.ascontiguousarray(part[:, :, 128:130]).view(
            BFDT).astype(np.float32)               # [NCH,128,1]
        np.multiply(part[:, :, 0:128], sc, out=out_view[p])
        _say(f"done {p}")

    def _pair(p):
        jax = state["jax"]
        dev = state["dev"]
        qkv_dev = jax.device_put(in_view[p].reshape(3, PPC, L, 130), dev)
        h = p % H
        whc = state["whc"]
        if state.get("whseq") != state["seq"]:
            whc.clear()
            state["whseq"] = state["seq"]
        if h not in whc:
            whc[h] = jax.device_put(
                np.ascontiguousarray(wh_view[h][None]).astype(BFDT), dev)
        o = state["fn"](qkv_dev, whc[h], *state["zeros"])[0]
        pool.submit(_fetch, o, p)

    _say("ready")
    try:
        for line in sys.stdin:
            parts = line.split()
            if not parts:
                continue
            if parts[0] == "init":
                try:
                    _init(float(parts[1]))
                    _say("inited")
                except Exception:
                    import traceback
                    traceback.print_exc(file=sys.stderr)
                    _say("initfail")
            elif parts[0] == "call":
                state["seq"] = int(parts[1])
            elif parts[0] == "pair":
                _pair(int(parts[1]))
            elif parts[0] == "quit":
                break
    finally:
        pool.shutdown(wait=False)


# ----------------------------------------------------------------------
# parent-side multiprocess driver
# ----------------------------------------------------------------------

_MP = {"procs": None}


class _WorkerHandle:
    def __init__(self, proc):
        self.proc = proc
        self.ready = threading.Event()
        self.inited = threading.Event()
        self.initfail = False
        self.dead = False
        self.dones = 0
        self.cond = threading.Condition()
        t = threading.Thread(target=self._reader, daemon=True)
        t.start()

    def _reader(self):
        for line in self.proc.stdout:
            s = line.strip()
            if s == "ready":
                self.ready.set()
            elif s == "inited":
                self.inited.set()
            elif s == "initfail":
                self.initfail = True
                self.inited.set()
            elif s.startswith("done "):
                with self.cond:
                    self.dones += 1
                    self.cond.notify()
        self.dead = True
        self.ready.set()
        self.inited.set()
        with self.cond:
            self.cond.notify()

    def send(self, msg):
        self.proc.stdin.write(msg + "\n")
        self.proc.stdin.flush()


def _mp_ensure():
    import subprocess
    from multiprocessing import shared_memory

    if _MP["procs"] is not None:
        return
    tag = f"bsk{os.getpid()}"
    shm_in = shared_memory.SharedMemory(
        create=True, size=PAIRS * 3 * L * 130, name=f"{tag}i")
    shm_out = shared_memory.SharedMemory(
        create=True, size=PAIRS * L * D * 4, name=f"{tag}o")
    shm_wh = shared_memory.SharedMemory(
        create=True, size=H * 128 * F * 4, name=f"{tag}w")
    procs = []
    for wi in range(NW):
        code = (
            "import sys; sys.path.insert(0, %r); import kernel; "
            "kernel._worker_main(%d, %r, %r, %r)"
            % (_KDIR, wi, f"{tag}i", f"{tag}o", f"{tag}w"))
        try:
            errdst = sys.stderr if sys.stderr.fileno() >= 0 else None
        except Exception:
            errdst = subprocess.DEVNULL
        p = subprocess.Popen(
            [sys.executable, "-u", "-c", code],
            stdin=subprocess.PIPE, stdout=subprocess.PIPE,
            stderr=errdst, text=True)
        procs.append(_WorkerHandle(p))
    _MP.update(procs=procs, shm_in=shm_in, shm_out=shm_out, shm_wh=shm_wh,
               in_view=np.ndarray((PAIRS, 3, L, 130), np.int8,
                                  buffer=shm_in.buf),
               out_view=np.ndarray((PAIRS, NCH, 128, 128), np.float32,
                                   buffer=shm_out.buf),
               wh_view=np.ndarray((H, 128, F), np.float32,
                                  buffer=shm_wh.buf),
               w=None, seq=0)


def _mp_run(q, k, v, wts, w):
    _mp_ensure()
    procs = _MP["procs"]
    for h in procs:
        if not h.ready.wait(timeout=300) or h.dead:
            raise RuntimeError("worker failed to start")
    if _MP["w"] != w:
        # worker 0 first: its compile populates the terminal's staged
        # executable cache, so the other workers' inits hit it and load
        # in seconds instead of recompiling
        for grp_ in (procs[:1], procs[1:]):
            for h in grp_:
                h.send(f"init {w!r}")
            for h in grp_:
                if not h.inited.wait(timeout=900) or h.initfail or h.dead:
                    raise RuntimeError("worker init failed")
                h.inited.clear()
        _MP["w"] = w
    _MP["seq"] += 1
    seq = _MP["seq"]
    _MP["wh_view"][:] = wts
    for h in procs:
        h.send(f"call {seq}")
    in_view = _MP["in_view"]
    qf = q.reshape(PAIRS, L, D)
    kf = k.reshape(PAIRS, L, D)
    vf = v.reshape(PAIRS, L, D)
    for p in range(PAIRS):
        _pack_pair_into(in_view[p], qf[p], kf[p], vf[p])
        procs[p % NW].send(f"pair {p}")
    deadline = time.time() + 300
    per_w = PAIRS // NW
    for h in procs:
        with h.cond:
            while h.dones < per_w and not h.dead:
                if not h.cond.wait(timeout=max(0.1, deadline - time.time())):
                    raise RuntimeError("worker timed out")
            if h.dead and h.dones < per_w:
                raise RuntimeError("worker died")
            h.dones = 0
    return _MP["out_view"].reshape(B, H, L, D).copy()


# ----------------------------------------------------------------------
# single-process fallback (8-core shard_map), and host reference
# ----------------------------------------------------------------------

_STATE = {}


def _build_state(w: float):
    import jax
    from jax.sharding import Mesh, PartitionSpec, NamedSharding
    from jax.experimental.shard_map import shard_map
    from concourse.bass2jax import install_neuronx_cc_hook

    nc = build_nc(w)
    install_neuronx_cc_hook()
    pn, in_names, out_names, out_avals = _collect_io(nc)
    body = _make_body(nc, pn, in_names, out_names, out_avals)
    n_params = len(in_names)
    n_outs = len(out_names)

    devices = jax.devices()[:NCORES]
    mesh = Mesh(np.asarray(devices), ("core",))
    spec = NamedSharding(mesh, PartitionSpec("core"))
    fn = jax.jit(
        shard_map(body, mesh=mesh,
                  in_specs=(PartitionSpec("core"),) * (n_params + n_outs),
                  out_specs=(PartitionSpec("core"),) * n_outs,
                  check_rep=False),
        keep_unused=True,
    )
    zeros = [
        jax.device_put(
            np.zeros((NCORES * a.shape[0], *a.shape[1:]), a.dtype), spec)
        for a in out_avals
    ]
    return {"fn": fn, "spec": spec, "zeros": zeros}


def _get_state(w: float):
    key = round(w, 10)
    if key not in _STATE:
        _STATE[key] = _build_state(w)
    return _STATE[key]


def _pack_qkv(q, k, v, j):
    sl = slice(j * RPC, (j + 1) * RPC)
    pack = np.empty((NCORES, 3, PPC, L, 130), dtype=np.int8)
    for t, x in enumerate((q, k, v)):
        xx = x[sl]
        s = (np.abs(xx).max(-1) / 126.0 + 1e-30).astype(BFDT)
        inv = 1.0 / s.astype(np.float32)
        tq = xx * inv[..., None]
        np.rint(tq, out=tq)
        dst = pack[:, t]
        dst[:, :, :, 0:128] = tq.reshape(NCORES, PPC, L, D)
        dst[:, :, :, 128:130] = s.view(np.int8).reshape(NCORES, PPC, L, 2)
    return pack.reshape(NCORES * 3, PPC, L, 130)


def _run_device(q, k, v, wts, w):
    import math

    import jax
    from concurrent.futures import ThreadPoolExecutor

    st = _get_state(w)
    qf = q.reshape(PAIRS, L, D)
    kf = k.reshape(PAIRS, L, D)
    vf = v.reshape(PAIRS, L, D)
    period = H // math.gcd(H, RPC)
    wh_devs = None

    fetches = []
    res = np.empty((PAIRS, NCH, 128, D), dtype=np.float32)

    def _fetch_into(out_arr, j):
        part = np.asarray(out_arr)            # [RPC, NCH, 128, 130] int8
        sc = np.ascontiguousarray(part[..., 128:130]).view(
            BFDT).astype(np.float32)
        sl = slice(j * RPC, (j + 1) * RPC)
        res[sl] = part[..., 0:128]
        res[sl] *= sc

    with ThreadPoolExecutor(max_workers=4) as ex:
        for j in range(NCALL):
            qkv_dev = jax.device_put(_pack_qkv(qf, kf, vf, j), st["spec"])
            if wh_devs is None:
                wh_devs = [
                    jax.device_put(
                        np.asarray(wts[(jj * RPC + np.arange(RPC)) % H],
                                   dtype=BFDT),
                        st["spec"])
                    for jj in range(period)
                ]
            o = st["fn"](qkv_dev, wh_devs[j % period], *st["zeros"])[0]
            fetches.append(ex.submit(_fetch_into, o, j))
        for f in fetches:
            f.result()

    return res.reshape(B, H, L, D)


def kernel(query_states, key_states, value_states, hedgehog_weights, alpha):
    q = np.asarray(query_states, dtype=np.float32)
    k = np.asarray(key_states, dtype=np.float32)
    v = np.asarray(value_states, dtype=np.float32)
    wts = np.asarray(hedgehog_weights, dtype=np.float32)
    a = float(np.asarray(alpha))
    w = float(1.0 / (1.0 + np.exp(-a)))

    if _MP.get("ok", True):
        try:
            return _mp_run(q, k, v, wts, w)
        except Exception:
            import traceback
            traceback.print_exc(file=sys.stderr)
            _MP["ok"] = False
    try:
        return _run_device(q, k, v, wts, w)
    except Exception:
        import traceback
        traceback.print_exc(file=sys.stderr)
        return _host_reference(q, k, v, wts, w)


def _host_reference(q, k, v, wts, w):
    # Last-resort fallback so a transient device failure still returns
    # a correct result; mirrors the block-scan math in fp32 numpy.
    out = np.empty((B, H, L, D), dtype=np.float32)
    for b in range(B):
        for h in range(H):
            u = q[b, h].reshape(NBLK, SBLK, D) @ wts[h]
            pq = np.concatenate([_sm(u), _sm(-u)], -1)
            uk = k[b, h].reshape(NBLK, SBLK, D) @ wts[h]
            pk = np.concatenate([_sm(uk), _sm(-uk)], -1)
            vb = v[b, h].reshape(NBLK, SBLK, D)
            qb = q[b, h].reshape(NBLK, SBLK, D)
            kb = k[b, h].reshape(NBLK, SBLK, D)
            S = np.zeros((2 * F, D), np.float32)
            Z = np.zeros((2 * F,), np.float32)
            for n in range(NBLK):
                den = np.maximum(pq[n] @ Z, EPS)
                lin = (pq[n] @ S) / den[:, None]
                S = S + pk[n].T @ vb[n]
                Z = Z + pk[n].sum(0)
                sc = qb[n] @ kb[n].T * SCALING
                p = _sm(sc)
                out[b, h, n * SBLK:(n + 1) * SBLK] = (
                    w * (p @ vb[n]) + (1 - w) * lin)
    return out


def _sm(x):
    e = np.exp(x - x.max(-1, keepdims=True))
    return e / e.sum(-1, keepdims=True)
